# revision 37
# baseline (speedup 1.0000x reference)
"""AttentionWithRoPE Trainium2 kernel (8-core SPMD), all-bf16 PE pipeline.

Sharding: core c handles batch b = c // 2 and head-group g = c % 2
(heads 4g..4g+3).  Each core computes rmsnorm(x_b), its 4 heads' Q/K/V
projections, RoPE, full-sequence attention, and a partial output
projection (its heads' rows of w_out).  Host sums the two partial
outputs per batch.

All matmul operands are bf16 (fp32 weights can't use the PE background
weight buffer, so their LDWEIGHTS serialize with the matmuls and the
HAM clock never warms).  RMS-norm runs on ACT (Square + Rsqrt — both in
resident table sets, one switch to Exp after).  Softmax exp is split
between ACT (spline exp) and DVE (Schraudolph bit-trick exp: one
tensor_scalar to int16, bitcast to bf16) so neither engine is the
bottleneck.  The denominator comes from a ones column appended to V
(attn@V row 64).  Logit matmuls contract 64 partitions, so the two
heads of a pair are issued adjacently at base partitions 0/64 to run
concurrently in separate PE row-groups.
"""

import math
import os
from contextlib import ExitStack

import numpy as np

import concourse.bass as bass
import concourse.tile as tile
from concourse import bacc, mybir

B, N, DIM = 4, 2048, 512
H, D = 8, 64
ROPE_THETA = 10000.0
NCORES = 8
SCALE = D ** -0.5

F32 = mybir.dt.float32
BF16 = mybir.dt.bfloat16
I16 = mybir.dt.int16

# Schraudolph exp in bf16-bits domain: e ~= bitcast_bf16(int16(A2*x + B2))
A2 = 128.0 / math.log(2.0)
B2 = 16250.5

# kt indices whose exp tiles run on DVE instead of ACT (tunable)
_dve_env = os.environ.get("KERNEL_DVE_KTS", "2,6,10,14")
DVE_KTS = set(int(t) for t in _dve_env.split(",") if t != "")


def build_program():
    nc = bacc.Bacc("TRN2", target_bir_lowering=False, debug=False)

    xT = nc.dram_tensor("xT", [DIM, N], F32, kind="ExternalInput").ap()
    wqk = nc.dram_tensor("wqk", [DIM, 512], BF16, kind="ExternalInput").ap()
    wv = nc.dram_tensor("wv", [DIM, 256], BF16, kind="ExternalInput").ap()
    wo = nc.dram_tensor("wo", [256, DIM], BF16, kind="ExternalInput").ap()
    cos2 = nc.dram_tensor("cos2", [128, N], BF16, kind="ExternalInput").ap()
    sinF2 = nc.dram_tensor("sinF2", [128, N], BF16, kind="ExternalInput").ap()
    ones_d = nc.dram_tensor("ones", [128, 128], BF16, kind="ExternalInput").ap()
    vones = nc.dram_tensor("vones", [128, 260], BF16, kind="ExternalInput").ap()
    yT = nc.dram_tensor("yT", [DIM, N], F32, kind="ExternalOutput").ap()

    DEBUG_TAPS = bool(os.environ.get("KERNEL_DEBUG_TAPS"))
    taps = {}
    if DEBUG_TAPS:
        for nm, shape, dt in [
            ("t_sinv", [128, N], F32),
            ("t_xn0", [128, N], BF16),
            ("t_qr0", [128, N], BF16),
            ("t_kr0", [128, N], BF16),
            ("t_v0", [128, 260], BF16),
            ("t_od00", [128, 1024], BF16),
        ]:
            taps[nm] = nc.dram_tensor(nm, shape, dt, kind="ExternalOutput").ap()

    AF = mybir.ActivationFunctionType

    with tile.TileContext(nc) as tc:
        with ExitStack() as ctx:
            persist = ctx.enter_context(tc.tile_pool(name="persist", bufs=1))
            work = ctx.enter_context(tc.tile_pool(name="work", bufs=3))
            rwork = ctx.enter_context(tc.tile_pool(name="rwork", bufs=2))
            ps_s = ctx.enter_context(tc.tile_pool(name="ps_s", bufs=2, space="PSUM"))
            ps_o = ctx.enter_context(tc.tile_pool(name="ps_o", bufs=4, space="PSUM"))
            exps = ctx.enter_context(tc.tile_pool(name="exps", bufs=6))
            aff = ctx.enter_context(tc.tile_pool(name="aff", bufs=2))
            rcp = ctx.enter_context(tc.tile_pool(name="rcp", bufs=2))
            ysb = ctx.enter_context(tc.tile_pool(name="ysb", bufs=2))

            # ---- small constants first, then x^T, then bulky v ones ----
            ones128 = persist.tile([128, 128], BF16, tag="ones128", name="ones128")
            nc.sync.dma_start(ones128[:], ones_d)
            wqk_t = []
            for i in range(4):
                t = persist.tile([128, 512], BF16, tag=f"wqk{i}", name=f"wqk{i}")
                nc.sync.dma_start(t[:], wqk[i * 128:(i + 1) * 128, :])
                wqk_t.append(t)
            cos_t = persist.tile([128, N], BF16, tag="cos", name="cos")
            nc.sync.dma_start(cos_t[:], cos2)
            sin_t = persist.tile([128, N], BF16, tag="sin", name="sin")
            nc.sync.dma_start(sin_t[:], sinF2)
            wv_t = []
            for i in range(4):
                t = persist.tile([128, 256], BF16, tag=f"wv{i}", name=f"wv{i}")
                nc.sync.dma_start(t[:], wv[i * 128:(i + 1) * 128, :])
                wv_t.append(t)
            wo_t = []
            for p in range(2):
                t = persist.tile([128, 512], BF16, tag=f"wo{p}", name=f"wo{p}")
                nc.sync.dma_start(t[:], wo[p * 128:(p + 1) * 128, :])
                wo_t.append(t)
            xt = []
            for i in range(4):
                t = persist.tile([128, N], F32, tag=f"xt{i}", name=f"xt{i}")
                nc.sync.dma_start(t[:], xT[i * 128:(i + 1) * 128, :])
                xt.append(t)
            v_sb = []
            for tt in range(16):
                t = persist.tile([128, 260], BF16, tag=f"v{tt}", name=f"v{tt}")
                nc.sync.dma_start(t[:], vones)
                v_sb.append(t)

            # ---- phase A: rmsnorm (ACT squares + sqrt), xn bf16 ----
            sinv = persist.tile([128, N], F32, tag="sinv", name="sinv")
            for c in range(4):
                cs = slice(c * 512, (c + 1) * 512)
                ss = ps_s.tile([128, 512], F32, tag="sc", name="ss")
                for i in range(4):
                    xsq = work.tile([128, 512], BF16, tag="xsq", name="xsq",
                                    bufs=3)
                    nc.scalar.activation(xsq[:], xt[i][:, cs], AF.Square)
                    nc.tensor.matmul(ss[:], ones128[:], xsq[:],
                                     start=(i == 0), stop=(i == 3))
                # snorm = sqrt(sumsq/512)  ->  1/snorm = sqrt(512)/||x||
                sn = work.tile([128, 512], F32, tag="snorm", name="snorm")
                nc.scalar.activation(sn[:], ss[:], AF.Sqrt, scale=1.0 / DIM)
                nc.vector.reciprocal_approx_fast(sinv[:, cs], sn[:])
            xn = []
            for i in range(4):
                t = persist.tile([128, N], BF16, tag=f"xn{i}", name=f"xn{i}")
                nc.vector.tensor_mul(t[:], xt[i][:], sinv[:])
                xn.append(t)

            # ---- Q/K projection + RoPE (whole m-tile at FD=2048) ----
            # wqk columns: [q h0..h3 | k h0..h3]; m=0: q pair0, m=1: q
            # pair1, m=2: k pair0, m=3: k pair1.  Head d-dims host-permuted
            # to [evens | odds] so the RoPE pair-swap is 32-row block moves.
            qk_dest = []
            for name in ["qr0", "qr1", "kr0", "kr1"]:
                t = persist.tile([128, N], BF16, tag=name, name=name)
                qk_dest.append(t)

            def emit_rope(m, cast_on_act=True):
                ms = slice(m * 128, (m + 1) * 128)
                qkb = rwork.tile([128, N], BF16, tag="qkb", name="qkb")
                for c in range(4):
                    cs = slice(c * 512, (c + 1) * 512)
                    qk = ps_s.tile([128, 512], F32, tag="sc", name="qkps")
                    for i in range(4):
                        nc.tensor.matmul(qk[:], wqk_t[i][:, ms],
                                         xn[i][:, cs],
                                         start=(i == 0), stop=(i == 3))
                    if cast_on_act:
                        nc.scalar.copy(qkb[:, cs], qk[:])
                    else:
                        nc.vector.tensor_copy(qkb[:, cs], qk[:])
                rotu = rwork.tile([128, N], BF16, tag="rotu", name="rotu")
                for h0 in (0, 64):
                    nc.vector.tensor_copy(rotu[h0:h0 + 32, :],
                                          qkb[h0 + 32:h0 + 64, :])
                    nc.vector.tensor_copy(rotu[h0 + 32:h0 + 64, :],
                                          qkb[h0:h0 + 32, :])
                dst = qk_dest[m]
                nc.vector.tensor_mul(dst[:], qkb[:], cos_t[:])
                nc.vector.tensor_mul(rotu[:], rotu[:], sin_t[:])
                nc.vector.tensor_add(dst[:], dst[:], rotu[:])

            # ---- V projection (token-major), ones column per head ----
            # v_sb layout: head j at cols [65j, 65j+64), col 65j+64 == 1.0
            def emit_v():
                for tt in range(16):
                    vp = ps_s.tile([128, 512], F32, tag="sc", name="vps")
                    ts = slice(tt * 128, (tt + 1) * 128)
                    for i in range(4):
                        nc.tensor.matmul(vp[:, 0:256], xn[i][:, ts], wv_t[i][:],
                                         start=(i == 0), stop=(i == 3))
                    dst = v_sb[tt][:].rearrange("p (h c) -> p h c", h=4)[:, :, 0:64]
                    nc.scalar.copy(dst, vp[:, 0:256].rearrange(
                        "p (h c) -> p h c", h=4))

            # ---- attention for one (query-half, head-pair) ----
            outd = [[None, None], [None, None]]

            def emit_attention(qq, hp):
                # query-quarter block (512 queries, head pair hp): each kt
                # gets ONE [128,1024] s tile holding both heads' logits
                # (j0 cols 0:512 -> bank 0, j1 cols 512:1024 -> bank 1).
                # The two 64-contract logit matmuls run concurrently in
                # different PE row groups; one exp covers both heads.
                # logits(kt+1) are emitted before attnV(kt) so the PE
                # pipelines past the exp latency.
                qr, kr = qk_dest[hp], qk_dest[2 + hp]
                qh, qsub = qq // 2, qq % 2
                qs = slice(qq * 512, (qq + 1) * 512)
                od = outd[hp][qh]
                if od is None:
                    od = persist.tile([128, 1024], BF16, tag=f"od{hp}{qh}",
                                      name=f"od{hp}{qh}")
                    outd[hp][qh] = od
                o_ps = [ps_o.tile([65, 512], F32, tag="o", name="o")
                        for _ in range(2)]

                def emit_attnv(kt, e):
                    for j in range(2):
                        h = 2 * hp + j
                        nc.tensor.matmul(
                            o_ps[j][:],
                            v_sb[kt][:, 65 * h:65 * h + 65],
                            e[:, j * 512:(j + 1) * 512],
                            start=(kt == 0), stop=(kt == 15),
                            skip_group_check=True)

                prev = None
                for kt in range(16):
                    ks = slice(kt * 128, (kt + 1) * 128)
                    s_ps = ps_s.tile([128, 1024], F32, tag="sc", name="sc")
                    for j in range(2):
                        js = slice(j * 64, (j + 1) * 64)
                        nc.tensor.matmul(
                            s_ps[:, j * 512:(j + 1) * 512],
                            kr[js, ks], qr[js, qs],
                            start=True, stop=True)
                    if kt in DVE_KTS:
                        taff = aff.tile([128, 1024], F32, tag="taff",
                                        name="taff")
                        nc.vector.tensor_scalar(
                            taff[:], s_ps[:], SCALE * A2, B2,
                            mybir.AluOpType.mult, mybir.AluOpType.add)
                        ei = exps.tile([128, 1024], I16, tag="e", name="e")
                        nc.vector.tensor_copy(ei[:], taff[:])
                        e = ei[:].bitcast(BF16)
                    else:
                        eb = exps.tile([128, 1024], BF16, tag="e", name="e")
                        nc.scalar.activation(eb[:], s_ps[:], AF.Exp,
                                             scale=SCALE)
                        e = eb[:]
                    if prev is not None:
                        emit_attnv(*prev)
                    prev = (kt, e)
                emit_attnv(*prev)

                ods = slice(qsub * 512, (qsub + 1) * 512)
                for j in range(2):
                    js = slice(j * 64, (j + 1) * 64)
                    rrow = rcp.tile([1, 512], F32, tag="rrow", name="rrow",
                                    bufs=1)
                    dcopy = rcp.tile([1, 512], F32, tag="dcopy", name="dcopy",
                                     bufs=1)
                    nc.vector.tensor_copy(dcopy[:], o_ps[j][64:65, :])
                    nc.vector.reciprocal_approx_fast(rrow[:], dcopy[:])
                    rfull = rcp.tile([64, 512], F32, tag="rfull", name="rfull")
                    nc.gpsimd.partition_broadcast(rfull[:], rrow[:])
                    nc.vector.tensor_mul(od[js, ods], o_ps[j][0:64, :],
                                         rfull[:])

            # ---- output projection for one query-half (partial w_out) ----
            def emit_proj(qh):
                for om in range(4):
                    oms = slice(om * 128, (om + 1) * 128)
                    yp = ps_s.tile([128, 1024], F32, tag="sc", name="yp")
                    for sub in range(2):
                        ss_ = slice(sub * 512, (sub + 1) * 512)
                        for p in range(2):
                            nc.tensor.matmul(
                                yp[:, ss_], wo_t[p][:, oms],
                                outd[p][qh][:, ss_],
                                start=(p == 0), stop=(p == 1))
                    yo = ysb.tile([128, 1024], F32, tag="y", name="y")
                    nc.vector.tensor_copy(yo[:], yp[:])
                    nc.sync.dma_start(
                        yT[oms, qh * 1024:(qh + 1) * 1024], yo[:])

            # ---- emission order: K and V first so attention can start
            # right after the first query chunks are roped ----
            emit_rope(2)      # kr0
            emit_rope(0)      # qr0
            emit_v()
            emit_attention(0, 0)
            emit_attention(1, 0)
            emit_rope(3, cast_on_act=False)      # kr1
            emit_rope(1, cast_on_act=False)      # qr1
            emit_attention(0, 1)
            emit_attention(1, 1)
            emit_proj(0)
            emit_attention(2, 0)
            emit_attention(3, 0)
            emit_attention(2, 1)
            emit_attention(3, 1)
            emit_proj(1)

            if DEBUG_TAPS:
                nc.sync.dma_start(taps["t_sinv"], sinv[:])
                nc.sync.dma_start(taps["t_xn0"], xn[0][:])
                nc.sync.dma_start(taps["t_qr0"], qk_dest[0][:])
                nc.sync.dma_start(taps["t_kr0"], qk_dest[2][:])
                nc.sync.dma_start(taps["t_v0"], v_sb[0][:])
                nc.sync.dma_start(taps["t_od00"], outd[0][0][:])

    nc.compile()
    return nc


def rope_tables():
    """cos / sign-folded sin tables in permuted ([evens | odds]) row order,
    stacked for two 64-row head slots.

    Device row r in [0, 32): holds d-dim 2r (even slot, rot sign -1);
    row r in [32, 64): d-dim 2(r-32)+1 (odd slot, rot sign +1).  Both use
    frequency index r % 32.
    """
    inv_freq = (1.0 / (ROPE_THETA ** (np.arange(0, D, 2, dtype=np.float32) / D)))
    freqs = np.arange(N, dtype=np.float32)[:, None] * inv_freq[None, :]  # [N, 32]
    cos = np.cos(freqs).T.astype(np.float32)  # [32, N]
    sin = np.sin(freqs).T.astype(np.float32)  # [32, N]
    cos64 = np.concatenate([cos, cos], axis=0)  # [64, N]
    sinF64 = np.concatenate([-sin, sin], axis=0)
    cos2 = np.concatenate([cos64, cos64], axis=0)  # [128, N]
    sinF2 = np.concatenate([sinF64, sinF64], axis=0)
    return np.ascontiguousarray(cos2), np.ascontiguousarray(sinF2)


_PERM64 = np.concatenate([np.arange(0, D, 2), np.arange(1, D, 2)])


def _permute_heads(w):
    """Permute each head's 64 columns of w [512, 256] to [evens | odds]."""
    w = w.reshape(DIM, 4, D)[:, :, _PERM64]
    return w.reshape(DIM, 256)


def _bf16():
    import ml_dtypes
    return ml_dtypes.bfloat16


def make_in_maps(x, gamma, w_qkv, w_out):
    bf = _bf16()
    cos2, sinF2 = rope_tables()
    wg = (gamma[:, None] * w_qkv).astype(np.float32)  # fold gamma
    in_maps = []
    for c in range(NCORES):
        b, g = c // 2, c % 2
        hs = slice(g * 256, (g + 1) * 256)
        wqk_c = np.concatenate([_permute_heads(wg[:, 0:512][:, hs]),
                                _permute_heads(wg[:, 512:1024][:, hs])],
                               axis=1)
        wv_c = wg[:, 1024:1536][:, hs]
        wo_c = w_out[hs, :]
        in_maps.append({
            "xT": np.ascontiguousarray(x[b].T).astype(np.float32),
            "wqk": np.ascontiguousarray(wqk_c).astype(bf),
            "wv": np.ascontiguousarray(wv_c).astype(bf),
            "wo": np.ascontiguousarray(wo_c).astype(bf),
            "cos2": cos2.astype(bf),
            "sinF2": sinF2.astype(bf),
            "ones": np.ones((128, 128), dtype=bf),
            "vones": np.ones((128, 260), dtype=bf),
        })
    return in_maps


_NC_CACHE = None


def _get_program():
    global _NC_CACHE
    if _NC_CACHE is None:
        _NC_CACHE = build_program()
    return _NC_CACHE


def run_cores(inputs, trace=False):
    """Run the SPMD kernel on 8 cores; returns (full_output, results)."""
    from concourse.bass_utils import run_bass_kernel_spmd

    nc = _get_program()
    in_maps = make_in_maps(inputs["x"], inputs["gamma"],
                           inputs["w_qkv"], inputs["w_out"])
    kwargs = {}
    if trace:
        _install_ntff_hook()
        kwargs = dict(trace=True, trace_cores=list(range(NCORES)))
    res = run_bass_kernel_spmd(nc, in_maps, core_ids=list(range(NCORES)),
                               **kwargs)
    out = np.empty((B, N, DIM), dtype=np.float32)
    for b in range(B):
        yTv = res.results[2 * b]["yT"] + res.results[2 * b + 1]["yT"]
        out[b] = yTv.T
    return out, res


def _install_ntff_hook():
    """Register the axon NTFF profiling hook (missing antenv.axon_hooks)."""
    import sys
    import types

    if "antenv.axon_hooks" in sys.modules:
        return
    try:
        import trn_agent_boot.trn_boot as tb
        import concourse.bass_utils as bu

        mod = types.ModuleType("antenv.axon_hooks")
        hook = tb._ntff_profile_via_ctypes("/opt/axon/libaxon_pjrt.so")
        mod.get_axon_ntff_profile_hook = lambda: hook
        sys.modules["antenv.axon_hooks"] = mod
        bu.upload_artifacts = lambda tmpdir: "local://" + tmpdir
    except Exception:
        pass


def kernel(**inputs):
    out, _ = run_cores(inputs, trace=bool(os.environ.get("KERNEL_TRACE")))
    return out


# revision 41
# speedup vs baseline: 1.0516x; 1.0516x over previous
"""AttentionWithRoPE Trainium2 kernel (8-core SPMD), all-bf16 PE pipeline.

Sharding: core c handles batch b = c // 2 and head-group g = c % 2
(heads 4g..4g+3).  Each core computes rmsnorm(x_b), its 4 heads' Q/K/V
projections, RoPE, full-sequence attention, and a partial output
projection (its heads' rows of w_out).  Host sums the two partial
outputs per batch.

All matmul operands are bf16 (fp32 weights can't use the PE background
weight buffer, so their LDWEIGHTS serialize with the matmuls and the
HAM clock never warms).  RMS-norm runs on ACT (Square + Rsqrt — both in
resident table sets, one switch to Exp after).  Softmax exp is split
between ACT (spline exp) and DVE (Schraudolph bit-trick exp: one
tensor_scalar to int16, bitcast to bf16) so neither engine is the
bottleneck.  The denominator comes from a ones column appended to V
(attn@V row 64).  Logit matmuls contract 64 partitions, so the two
heads of a pair are issued adjacently at base partitions 0/64 to run
concurrently in separate PE row-groups.
"""

import math
import os
from contextlib import ExitStack

import numpy as np

import concourse.bass as bass
import concourse.tile as tile
from concourse import bacc, mybir

B, N, DIM = 4, 2048, 512
H, D = 8, 64
ROPE_THETA = 10000.0
NCORES = 8
SCALE = D ** -0.5

F32 = mybir.dt.float32
BF16 = mybir.dt.bfloat16
I16 = mybir.dt.int16

# Schraudolph exp in bf16-bits domain: e ~= bitcast_bf16(int16(A2*x + B2))
A2 = 128.0 / math.log(2.0)
B2 = 16250.5

# kt indices whose exp tiles run on DVE instead of ACT (tunable)
_dve_env = os.environ.get("KERNEL_DVE_KTS", "2,6,10,14")
DVE_KTS = set(int(t) for t in _dve_env.split(",") if t != "")


def build_program():
    nc = bacc.Bacc("TRN2", target_bir_lowering=False, debug=False)

    xT = nc.dram_tensor("xT", [DIM, N], F32, kind="ExternalInput").ap()
    wqk = nc.dram_tensor("wqk", [DIM, 512], BF16, kind="ExternalInput").ap()
    wv = nc.dram_tensor("wv", [DIM, 256], BF16, kind="ExternalInput").ap()
    wo = nc.dram_tensor("wo", [256, DIM], BF16, kind="ExternalInput").ap()
    cos2 = nc.dram_tensor("cos2", [128, N], BF16, kind="ExternalInput").ap()
    sinF2 = nc.dram_tensor("sinF2", [128, N], BF16, kind="ExternalInput").ap()
    ones_d = nc.dram_tensor("ones", [128, 128], BF16, kind="ExternalInput").ap()
    vones = nc.dram_tensor("vones", [128, 260], BF16, kind="ExternalInput").ap()
    yT = nc.dram_tensor("yT", [DIM, N], F32, kind="ExternalOutput").ap()

    DEBUG_TAPS = bool(os.environ.get("KERNEL_DEBUG_TAPS"))
    taps = {}
    if DEBUG_TAPS:
        for nm, shape, dt in [
            ("t_sinv", [128, N], F32),
            ("t_xn0", [128, N], BF16),
            ("t_qr0", [128, N], BF16),
            ("t_kr0", [128, N], BF16),
            ("t_v0", [128, 260], BF16),
            ("t_od00", [128, 1024], BF16),
        ]:
            taps[nm] = nc.dram_tensor(nm, shape, dt, kind="ExternalOutput").ap()

    AF = mybir.ActivationFunctionType

    with tile.TileContext(nc) as tc:
        with ExitStack() as ctx:
            persist = ctx.enter_context(tc.tile_pool(name="persist", bufs=1))
            work = ctx.enter_context(tc.tile_pool(name="work", bufs=3))
            rwork = ctx.enter_context(tc.tile_pool(name="rwork", bufs=2))
            ps_s = ctx.enter_context(tc.tile_pool(name="ps_s", bufs=2, space="PSUM"))
            ps_o = ctx.enter_context(tc.tile_pool(name="ps_o", bufs=4, space="PSUM"))
            exps = ctx.enter_context(tc.tile_pool(name="exps", bufs=6))
            aff = ctx.enter_context(tc.tile_pool(name="aff", bufs=2))
            rcp = ctx.enter_context(tc.tile_pool(name="rcp", bufs=2))
            ysb = ctx.enter_context(tc.tile_pool(name="ysb", bufs=2))

            # ---- small constants first, then x^T, then bulky v ones ----
            ones128 = persist.tile([128, 128], BF16, tag="ones128", name="ones128")
            nc.sync.dma_start(ones128[:], ones_d)
            wqk_t = []
            for i in range(4):
                t = persist.tile([128, 512], BF16, tag=f"wqk{i}", name=f"wqk{i}")
                nc.sync.dma_start(t[:], wqk[i * 128:(i + 1) * 128, :])
                wqk_t.append(t)
            cos_t = persist.tile([128, N], BF16, tag="cos", name="cos")
            nc.sync.dma_start(cos_t[:], cos2)
            sin_t = persist.tile([128, N], BF16, tag="sin", name="sin")
            nc.sync.dma_start(sin_t[:], sinF2)
            wv_t = []
            for i in range(4):
                t = persist.tile([128, 256], BF16, tag=f"wv{i}", name=f"wv{i}")
                nc.sync.dma_start(t[:], wv[i * 128:(i + 1) * 128, :])
                wv_t.append(t)
            wo_t = []
            for p in range(2):
                t = persist.tile([128, 512], BF16, tag=f"wo{p}", name=f"wo{p}")
                nc.sync.dma_start(t[:], wo[p * 128:(p + 1) * 128, :])
                wo_t.append(t)
            # x^T in 512-column chunks (c-major) so the rmsnorm pipeline
            # starts after the first 4 chunks instead of the full 4 MB
            xt = [persist.tile([128, N], F32, tag=f"xt{i}", name=f"xt{i}")
                  for i in range(4)]
            for c in range(4):
                cs = slice(c * 512, (c + 1) * 512)
                for i in range(4):
                    nc.sync.dma_start(xt[i][:, cs],
                                      xT[i * 128:(i + 1) * 128, cs])
            v_sb = []
            for tt in range(16):
                t = persist.tile([128, 260], BF16, tag=f"v{tt}", name=f"v{tt}")
                nc.sync.dma_start(t[:], vones)
                v_sb.append(t)

            # ---- phase A: rmsnorm (ACT squares + sqrt), xn bf16,
            # pipelined per 512-column chunk ----
            sinv = persist.tile([128, N], F32, tag="sinv", name="sinv")
            xn = [persist.tile([128, N], BF16, tag=f"xn{i}", name=f"xn{i}")
                  for i in range(4)]
            for c in range(4):
                cs = slice(c * 512, (c + 1) * 512)
                ss = ps_s.tile([128, 512], F32, tag="sc", name="ss")
                for i in range(4):
                    xsq = work.tile([128, 512], BF16, tag="xsq", name="xsq",
                                    bufs=3)
                    nc.scalar.activation(xsq[:], xt[i][:, cs], AF.Square)
                    nc.tensor.matmul(ss[:], ones128[:], xsq[:],
                                     start=(i == 0), stop=(i == 3))
                # snorm = sqrt(sumsq/512)  ->  1/snorm = sqrt(512)/||x||
                sn = work.tile([128, 512], F32, tag="snorm", name="snorm")
                nc.scalar.activation(sn[:], ss[:], AF.Sqrt, scale=1.0 / DIM)
                nc.vector.reciprocal_approx_fast(sinv[:, cs], sn[:])
                for i in range(4):
                    nc.vector.tensor_mul(xn[i][:, cs], xt[i][:, cs],
                                         sinv[:, cs])

            # ---- Q/K projection + RoPE (whole m-tile at FD=2048) ----
            # wqk columns: [q h0..h3 | k h0..h3]; m=0: q pair0, m=1: q
            # pair1, m=2: k pair0, m=3: k pair1.  Head d-dims host-permuted
            # to [evens | odds] so the RoPE pair-swap is 32-row block moves.
            qk_dest = []
            for name in ["qr0", "qr1", "kr0", "kr1"]:
                t = persist.tile([128, N], BF16, tag=name, name=name)
                qk_dest.append(t)

            def emit_rope(m, cast_on_act=True):
                ms = slice(m * 128, (m + 1) * 128)
                qkb = rwork.tile([128, N], BF16, tag="qkb", name="qkb")
                for c in range(4):
                    cs = slice(c * 512, (c + 1) * 512)
                    qk = ps_s.tile([128, 512], F32, tag="sc", name="qkps")
                    for i in range(4):
                        nc.tensor.matmul(qk[:], wqk_t[i][:, ms],
                                         xn[i][:, cs],
                                         start=(i == 0), stop=(i == 3))
                    if cast_on_act:
                        nc.scalar.copy(qkb[:, cs], qk[:])
                    else:
                        nc.vector.tensor_copy(qkb[:, cs], qk[:])
                rotu = rwork.tile([128, N], BF16, tag="rotu", name="rotu")
                for h0 in (0, 64):
                    nc.vector.tensor_copy(rotu[h0:h0 + 32, :],
                                          qkb[h0 + 32:h0 + 64, :])
                    nc.vector.tensor_copy(rotu[h0 + 32:h0 + 64, :],
                                          qkb[h0:h0 + 32, :])
                dst = qk_dest[m]
                nc.vector.tensor_mul(dst[:], qkb[:], cos_t[:])
                nc.vector.tensor_mul(rotu[:], rotu[:], sin_t[:])
                nc.vector.tensor_add(dst[:], dst[:], rotu[:])

            # ---- V projection (token-major), ones column per head ----
            # v_sb layout: head j at cols [65j, 65j+64), col 65j+64 == 1.0
            def emit_v():
                for tt in range(16):
                    vp = ps_s.tile([128, 512], F32, tag="sc", name="vps")
                    ts = slice(tt * 128, (tt + 1) * 128)
                    for i in range(4):
                        nc.tensor.matmul(vp[:, 0:256], xn[i][:, ts], wv_t[i][:],
                                         start=(i == 0), stop=(i == 3))
                    dst = v_sb[tt][:].rearrange("p (h c) -> p h c", h=4)[:, :, 0:64]
                    src = vp[:, 0:256].rearrange("p (h c) -> p h c", h=4)
                    if tt % 2 == 0:
                        nc.scalar.copy(dst, src)
                    else:
                        nc.vector.tensor_copy(dst, src)

            # ---- attention for one (query-half, head-pair) ----
            outd = [[None, None], [None, None]]

            def emit_attention(qq, hp):
                # query-quarter block (512 queries, head pair hp): each kt
                # gets ONE [128,1024] s tile holding both heads' logits
                # (j0 cols 0:512 -> bank 0, j1 cols 512:1024 -> bank 1).
                # The two 64-contract logit matmuls run concurrently in
                # different PE row groups; one exp covers both heads.
                # logits(kt+1) are emitted before attnV(kt) so the PE
                # pipelines past the exp latency.
                qr, kr = qk_dest[hp], qk_dest[2 + hp]
                qh, qsub = qq // 2, qq % 2
                qs = slice(qq * 512, (qq + 1) * 512)
                od = outd[hp][qh]
                if od is None:
                    od = persist.tile([128, 1024], BF16, tag=f"od{hp}{qh}",
                                      name=f"od{hp}{qh}")
                    outd[hp][qh] = od
                o_ps = [ps_o.tile([65, 512], F32, tag="o", name="o")
                        for _ in range(2)]

                def emit_attnv(kt, e):
                    for j in range(2):
                        h = 2 * hp + j
                        nc.tensor.matmul(
                            o_ps[j][:],
                            v_sb[kt][:, 65 * h:65 * h + 65],
                            e[:, j * 512:(j + 1) * 512],
                            start=(kt == 0), stop=(kt == 15),
                            skip_group_check=True)

                prev = None
                for kt in range(16):
                    ks = slice(kt * 128, (kt + 1) * 128)
                    s_ps = ps_s.tile([128, 1024], F32, tag="sc", name="sc")
                    for j in range(2):
                        js = slice(j * 64, (j + 1) * 64)
                        nc.tensor.matmul(
                            s_ps[:, j * 512:(j + 1) * 512],
                            kr[js, ks], qr[js, qs],
                            start=True, stop=True)
                    if kt in DVE_KTS:
                        taff = aff.tile([128, 1024], F32, tag="taff",
                                        name="taff")
                        nc.vector.tensor_scalar(
                            taff[:], s_ps[:], SCALE * A2, B2,
                            mybir.AluOpType.mult, mybir.AluOpType.add)
                        ei = exps.tile([128, 1024], I16, tag="e", name="e")
                        nc.vector.tensor_copy(ei[:], taff[:])
                        e = ei[:].bitcast(BF16)
                    else:
                        eb = exps.tile([128, 1024], BF16, tag="e", name="e")
                        nc.scalar.activation(eb[:], s_ps[:], AF.Exp,
                                             scale=SCALE)
                        e = eb[:]
                    if prev is not None:
                        emit_attnv(*prev)
                    prev = (kt, e)
                emit_attnv(*prev)

                ods = slice(qsub * 512, (qsub + 1) * 512)
                for j in range(2):
                    js = slice(j * 64, (j + 1) * 64)
                    rrow = rcp.tile([1, 512], F32, tag="rrow", name="rrow",
                                    bufs=1)
                    dcopy = rcp.tile([1, 512], F32, tag="dcopy", name="dcopy",
                                     bufs=1)
                    nc.vector.tensor_copy(dcopy[:], o_ps[j][64:65, :])
                    nc.vector.reciprocal_approx_fast(rrow[:], dcopy[:])
                    rfull = rcp.tile([64, 512], F32, tag="rfull", name="rfull")
                    nc.gpsimd.partition_broadcast(rfull[:], rrow[:])
                    nc.vector.tensor_mul(od[js, ods], o_ps[j][0:64, :],
                                         rfull[:])

            # ---- output projection for one query-half (partial w_out) ----
            def emit_proj(qh):
                for om in range(4):
                    oms = slice(om * 128, (om + 1) * 128)
                    yp = ps_s.tile([128, 1024], F32, tag="sc", name="yp")
                    for sub in range(2):
                        ss_ = slice(sub * 512, (sub + 1) * 512)
                        for p in range(2):
                            nc.tensor.matmul(
                                yp[:, ss_], wo_t[p][:, oms],
                                outd[p][qh][:, ss_],
                                start=(p == 0), stop=(p == 1))
                    yo = ysb.tile([128, 1024], F32, tag="y", name="y")
                    nc.vector.tensor_copy(yo[:], yp[:])
                    nc.sync.dma_start(
                        yT[oms, qh * 1024:(qh + 1) * 1024], yo[:])

            # ---- emission order: K and V first so attention can start
            # right after the first query chunks are roped ----
            emit_rope(2)      # kr0
            emit_rope(0)      # qr0
            emit_v()
            emit_rope(3)      # kr1
            emit_rope(1)      # qr1
            emit_attention(0, 0)
            emit_attention(1, 0)
            emit_attention(0, 1)
            emit_attention(1, 1)
            emit_proj(0)
            emit_attention(2, 0)
            emit_attention(3, 0)
            emit_attention(2, 1)
            emit_attention(3, 1)
            emit_proj(1)

            if DEBUG_TAPS:
                nc.sync.dma_start(taps["t_sinv"], sinv[:])
                nc.sync.dma_start(taps["t_xn0"], xn[0][:])
                nc.sync.dma_start(taps["t_qr0"], qk_dest[0][:])
                nc.sync.dma_start(taps["t_kr0"], qk_dest[2][:])
                nc.sync.dma_start(taps["t_v0"], v_sb[0][:])
                nc.sync.dma_start(taps["t_od00"], outd[0][0][:])

    nc.compile()
    return nc


def rope_tables():
    """cos / sign-folded sin tables in permuted ([evens | odds]) row order,
    stacked for two 64-row head slots.

    Device row r in [0, 32): holds d-dim 2r (even slot, rot sign -1);
    row r in [32, 64): d-dim 2(r-32)+1 (odd slot, rot sign +1).  Both use
    frequency index r % 32.
    """
    inv_freq = (1.0 / (ROPE_THETA ** (np.arange(0, D, 2, dtype=np.float32) / D)))
    freqs = np.arange(N, dtype=np.float32)[:, None] * inv_freq[None, :]  # [N, 32]
    cos = np.cos(freqs).T.astype(np.float32)  # [32, N]
    sin = np.sin(freqs).T.astype(np.float32)  # [32, N]
    cos64 = np.concatenate([cos, cos], axis=0)  # [64, N]
    sinF64 = np.concatenate([-sin, sin], axis=0)
    cos2 = np.concatenate([cos64, cos64], axis=0)  # [128, N]
    sinF2 = np.concatenate([sinF64, sinF64], axis=0)
    return np.ascontiguousarray(cos2), np.ascontiguousarray(sinF2)


_PERM64 = np.concatenate([np.arange(0, D, 2), np.arange(1, D, 2)])


def _permute_heads(w):
    """Permute each head's 64 columns of w [512, 256] to [evens | odds]."""
    w = w.reshape(DIM, 4, D)[:, :, _PERM64]
    return w.reshape(DIM, 256)


def _bf16():
    import ml_dtypes
    return ml_dtypes.bfloat16


def make_in_maps(x, gamma, w_qkv, w_out):
    bf = _bf16()
    cos2, sinF2 = rope_tables()
    wg = (gamma[:, None] * w_qkv).astype(np.float32)  # fold gamma
    in_maps = []
    for c in range(NCORES):
        b, g = c // 2, c % 2
        hs = slice(g * 256, (g + 1) * 256)
        wqk_c = np.concatenate([_permute_heads(wg[:, 0:512][:, hs]),
                                _permute_heads(wg[:, 512:1024][:, hs])],
                               axis=1)
        wv_c = wg[:, 1024:1536][:, hs]
        wo_c = w_out[hs, :]
        in_maps.append({
            "xT": np.ascontiguousarray(x[b].T).astype(np.float32),
            "wqk": np.ascontiguousarray(wqk_c).astype(bf),
            "wv": np.ascontiguousarray(wv_c).astype(bf),
            "wo": np.ascontiguousarray(wo_c).astype(bf),
            "cos2": cos2.astype(bf),
            "sinF2": sinF2.astype(bf),
            "ones": np.ones((128, 128), dtype=bf),
            "vones": np.ones((128, 260), dtype=bf),
        })
    return in_maps


_NC_CACHE = None


def _get_program():
    global _NC_CACHE
    if _NC_CACHE is None:
        _NC_CACHE = build_program()
    return _NC_CACHE


def run_cores(inputs, trace=False):
    """Run the SPMD kernel on 8 cores; returns (full_output, results)."""
    from concourse.bass_utils import run_bass_kernel_spmd

    nc = _get_program()
    in_maps = make_in_maps(inputs["x"], inputs["gamma"],
                           inputs["w_qkv"], inputs["w_out"])
    kwargs = {}
    if trace:
        _install_ntff_hook()
        kwargs = dict(trace=True, trace_cores=list(range(NCORES)))
    res = run_bass_kernel_spmd(nc, in_maps, core_ids=list(range(NCORES)),
                               **kwargs)
    out = np.empty((B, N, DIM), dtype=np.float32)
    for b in range(B):
        yTv = res.results[2 * b]["yT"] + res.results[2 * b + 1]["yT"]
        out[b] = yTv.T
    return out, res


def _install_ntff_hook():
    """Register the axon NTFF profiling hook (missing antenv.axon_hooks)."""
    import sys
    import types

    if "antenv.axon_hooks" in sys.modules:
        return
    try:
        import trn_agent_boot.trn_boot as tb
        import concourse.bass_utils as bu

        mod = types.ModuleType("antenv.axon_hooks")
        hook = tb._ntff_profile_via_ctypes("/opt/axon/libaxon_pjrt.so")
        mod.get_axon_ntff_profile_hook = lambda: hook
        sys.modules["antenv.axon_hooks"] = mod
        bu.upload_artifacts = lambda tmpdir: "local://" + tmpdir
    except Exception:
        pass


def kernel(**inputs):
    out, _ = run_cores(inputs, trace=bool(os.environ.get("KERNEL_TRACE")))
    return out
